# revision 9
# baseline (speedup 1.0000x reference)
"""KV page-cache scatter update on 8 Trainium2 NeuronCores.

Strategy (paged-attention style): shard kv_pages along the page axis —
128 pages per core.  On the host, route each valid token to the core
owning its destination page and build a dense per-core payload of the
routed tokens' combined K||V rows (one slot = 16*128 f32 = 8KB
contiguous; K is the first 4KB, V the second), bucketed by destination
copy-chunk.  Each core then:
  1. bulk-copies its kv_pages shard to the output shard (HWDGE DMA,
     DRAM->DRAM, chunks striped across both HWDGE rings),
  2. loads the routed payload into SBUF (contiguous SWDGE DMA),
  3. scatters the 8KB rows into the output shard with indirect DMA.
     Scatter group j holds only tokens whose dest lies inside copy chunk
     c_j, so it waits on that single chunk's semaphore and overlaps the
     rest of the bulk copy.

Each copy chunk gets its own semaphore: same-ring DMAs complete out of
order across the 16 SDMA engines, but each engine drains its ring FIFO,
so chunk c's sem reaching 16 proves the chunk (and all earlier same-ring
chunks) is done.  Padding entries point at slot index SLOTS, dropped by
the scatter's bounds check.
"""

import os
from contextlib import ExitStack

import numpy as np

import concourse.bass as bass
import concourse.mybir as mybir
from concourse.bass import IndirectOffsetOnAxis
from concourse.bass_utils import run_bass_kernel_spmd

NUM_PAGES = 1024
PAGE_SIZE = 64
KV_HEADS = 8
HEAD_DIM = 128
NUM_TOKENS = 8192

N_CORES = 8
PAGES_PER_CORE = NUM_PAGES // N_CORES          # 128
SLOTS = PAGES_PER_CORE * PAGE_SIZE             # 8192 slots per core
ROW = 2 * KV_HEADS * HEAD_DIM                  # 2048 f32 per slot (8KB)
HALF = KV_HEADS * HEAD_DIM                     # 1024 f32 (4KB)
GRP = 128                                      # max tokens per scatter group

# Pad sentinel: one past the last valid slot — fails the bounds check so the
# scatter drops it, and idx*row_stride stays far below int32 overflow.
DROP = np.int32(SLOTS)

LAST_RESULTS = None  # set by kernel(); lets test.py read exec_time_ns


def build_nc(subs: tuple, n_chunk: int, slots: int = SLOTS, row: int = ROW,
             grp: int = GRP, split_copy: bool = True):
    """Build the per-core SPMD Bass program.

    subs: tuple of (chunk_idx, width) — scatter group j holds `width`
    tokens whose dests all lie in copy chunk `chunk_idx`'s slot range.

    Inputs (per core): kv [slots,row] shard, kvr [sum(widths),row] routed
    dense K||V payload (group blocks concatenated), di [grp,n_subs] i32
    dest slots (group j in column j).  Output: out [slots,row].
    """
    f32 = mybir.dt.float32
    i32 = mybir.dt.int32
    n_subs = len(subs)
    total_rows = sum(w for _, w in subs)
    nc = bass.Bass()
    kv = nc.declare_dram_parameter("kv", [slots, row], f32, isOutput=False)
    kvr = nc.declare_dram_parameter("kvr", [total_rows, row], f32,
                                    isOutput=False)
    di = nc.declare_dram_parameter("di", [grp, n_subs], i32, isOutput=False)
    out = nc.declare_dram_parameter("out", [slots, row], f32, isOutput=True)

    chunk_rows = slots // n_chunk
    ring_of = (lambda i: i % 2) if split_copy else (lambda i: 0)

    with ExitStack() as ctx:
        kvt = ctx.enter_context(nc.sbuf_tensor([grp, n_subs * row], f32))
        di_sb = ctx.enter_context(nc.sbuf_tensor([grp, n_subs], i32))
        chunk_sems = [
            ctx.enter_context(nc.semaphore(f"chunk_sem{i}")) for i in range(n_chunk)
        ]
        idx_sem = ctx.enter_context(nc.semaphore("idx_sem"))
        load_sem = ctx.enter_context(nc.semaphore("load_sem"))
        scat_sem = ctx.enter_context(nc.semaphore("scat_sem"))
        block = ctx.enter_context(nc.Block())

        @block.sync
        def _(sync):
            for i in range(n_chunk):
                if ring_of(i) != 0:
                    continue
                r = slice(i * chunk_rows, (i + 1) * chunk_rows)
                sync.dma_start(out=out[r, :], in_=kv[r, :]).then_inc(
                    chunk_sems[i], 16)

        if split_copy:
            @block.scalar
            def _(sc):
                for i in range(n_chunk):
                    if ring_of(i) != 1:
                        continue
                    r = slice(i * chunk_rows, (i + 1) * chunk_rows)
                    sc.dma_start(out=out[r, :], in_=kv[r, :]).then_inc(
                        chunk_sems[i], 16)

        @block.gpsimd
        def _(g):
            g.dma_start(out=di_sb[:, :], in_=di[:, :]).then_inc(idx_sem, 16)
            r0 = 0
            for j, (_, w) in enumerate(subs):
                g.dma_start(
                    out=kvt[:w, j * row : (j + 1) * row],
                    in_=kvr[r0 : r0 + w, :],
                ).then_inc(load_sem, 16)
                r0 += w
            g.wait_ge(idx_sem, 16)
            g.wait_ge(load_sem, 16 * n_subs)
            for j, (c, w) in enumerate(subs):
                g.wait_ge(chunk_sems[c], 16)
                g.indirect_dma_start(
                    out=out[:, :],
                    out_offset=IndirectOffsetOnAxis(ap=di_sb[:w, j : j + 1], axis=0),
                    in_=kvt[:w, j * row : (j + 1) * row],
                    in_offset=None,
                    bounds_check=slots - 1,
                    oob_is_err=False,
                ).then_inc(scat_sem, 16)
            # drain: newest chunk of each ring + all scatters
            for ring in (0, 1):
                last = [i for i in range(n_chunk) if ring_of(i) == ring]
                if last:
                    g.wait_ge(chunk_sems[last[-1]], 16)
            g.wait_ge(scat_sem, n_subs * 16)

    return nc


_cache = {}


def _get_nc(subs: tuple, n_chunk: int, split_copy: bool):
    key = (subs, n_chunk, split_copy)
    if key not in _cache:
        _cache[key] = build_nc(subs, n_chunk, split_copy=split_copy)
    return _cache[key]


def _route(token_dests: np.ndarray, kn: np.ndarray, vn: np.ndarray,
           n_chunk: int):
    """Host-side routing: per core, bucket tokens by dest copy-chunk and
    build the dense K||V payload per scatter group.

    Returns (kvr [N_CORES,total_rows,ROW], di [N_CORES,GRP,n_subs], subs).
    subs[j] = (chunk_idx, width): width = max token count in that chunk's
    slot range across cores (split into <=GRP pieces), so group j has the
    same shape on every core; cores with fewer tokens pad with DROP."""
    chunk_rows = SLOTS // n_chunk
    dests = token_dests.astype(np.int64)
    valid = np.nonzero(dests >= 0)[0]
    d = dests[valid]
    core = d // SLOTS
    local = d - core * SLOTS
    chunk = local // chunk_rows

    # tokens per (core, chunk), sorted by slot within the bucket
    buckets = {}
    counts = np.zeros((N_CORES, n_chunk), np.int64)
    for c in range(N_CORES):
        selc = np.nonzero(core == c)[0]
        for ch in range(n_chunk):
            sel = selc[chunk[selc] == ch]
            sel = sel[np.argsort(local[sel], kind="stable")]
            buckets[(c, ch)] = sel
            counts[c, ch] = len(sel)

    caps = counts.max(axis=0)                      # per-chunk width needed
    subs = []
    for ch in range(n_chunk):
        cap = int(caps[ch])
        while cap > 0:
            w = min(cap, GRP)
            subs.append((ch, max(w, 2)))           # w>=2: offset AP can't be [1,1]
            cap -= w
    subs = tuple(subs)

    total_rows = sum(w for _, w in subs)
    kvr = np.zeros((N_CORES, total_rows, ROW), np.float32)
    di = np.full((N_CORES, GRP, len(subs)), DROP, np.int32)
    for c in range(N_CORES):
        used = {ch: 0 for ch in range(n_chunk)}
        r0 = 0
        for j, (ch, w) in enumerate(subs):
            sel = buckets[(c, ch)][used[ch] : used[ch] + w]
            used[ch] += w
            n = len(sel)
            if n:
                kvr[c, r0 : r0 + n, :HALF] = kn[valid[sel]]
                kvr[c, r0 : r0 + n, HALF:] = vn[valid[sel]]
                di[c, :n, j] = local[sel]
            r0 += w
    return kvr, di, subs


def kernel(kv_pages: np.ndarray, new_k: np.ndarray, new_v: np.ndarray,
           token_dests: np.ndarray) -> np.ndarray:
    global LAST_RESULTS
    kv_pages = np.ascontiguousarray(np.asarray(kv_pages, np.float32))
    kn = np.asarray(new_k, np.float32).reshape(NUM_TOKENS, HALF)
    vn = np.asarray(new_v, np.float32).reshape(NUM_TOKENS, HALF)
    token_dests = np.asarray(token_dests)

    n_chunk = int(os.environ.get("KV_NCHUNK", "16"))
    split_copy = os.environ.get("KV_SPLIT_COPY", "1") == "1"
    kvr, di, subs = _route(token_dests, kn, vn, n_chunk)
    nc = _get_nc(subs, n_chunk, split_copy)

    kv_flat = kv_pages.reshape(N_CORES, SLOTS, ROW)
    in_maps = [
        {"kv": kv_flat[c], "kvr": kvr[c], "di": di[c]}
        for c in range(N_CORES)
    ]
    res = run_bass_kernel_spmd(nc, in_maps, list(range(N_CORES)))
    LAST_RESULTS = res
    out = np.concatenate([res.results[c]["out"][None] for c in range(N_CORES)], axis=0)
    return out.reshape(NUM_PAGES, PAGE_SIZE, 2 * KV_HEADS, HEAD_DIM)
